# revision 19
# baseline (speedup 1.0000x reference)
"""Causal multi-head attention block (qkv proj + attention + out proj) on 8
Trainium2 NeuronCores.

Sharding: core c = 2*b + hg handles batch b (of 4) and head-group hg (8 of 16
heads).  Each core computes qkv for its heads, causal attention, and a partial
out-projection (its 512 rows of w_out); the host sums the two head-group
partials per batch.

Design:
  - host pre-transposes x to xT [DIM, T] and pre-casts to bf16: no PE
    transposes on device, half the input DMA bytes.
  - qkv projection, scores and out_proj run in bf16 (1 cycle/row at every
    p-state and free size).  P@V runs in fp8e4m3 for q-blocks 1-3 — DoubleRow
    perf mode on off-diagonal k-tile pairs (two 128-row contraction slabs per
    matmul at 0.5 cycles/row) and plain fp8 on diagonal tiles — and in bf16
    for q-block 0, whose short attention rows would amplify fp8 quantization
    past the error budget (verified by simulation: hybrid 4.3e-3 vs 2.8e-2
    for fp8-everywhere, gate 2e-2).
  - V is stored 16x-scaled in the fp8 tiles (subnormal dodge) with a 16.0
    ones column, which cancels exactly in the softmax ratio; exp applies
    bias -3.25 so the max P (max causal score is 66.5 -> logit 8.3) stays
    under fp8e4m3's 240.
  - softmax normalization: DVE copies evacuate the PSUM accumulator fast,
    the reciprocal runs 8 elems/lane via a DRAM reshape on the idle gpsimd
    SWDGE queue, and the divide doubles as the f32->bf16 cast for out_proj.
  - emission is phase-interleaved: qkv quarter q+1 / out_proj q-1 (dense PE
    work) pump into attention block q (ACT-heavy) to keep the PE dense and
    p-state ramped; input DMA is split across both HWDGE queues with
    first-needed tiles first.
"""

import sys

if "/opt/trn_rl_repo" not in sys.path:
    sys.path.insert(0, "/opt/trn_rl_repo")

import numpy as np
import ml_dtypes

import concourse.bass as bass
import concourse.mybir as mybir
import concourse.tile as tile
from concourse import bacc
from concourse.bass_utils import run_bass_kernel_spmd

DIM = 1024
N_HEAD = 16
HD = 64
B, T = 4, 2048
HG = 8          # heads per core
CQ = HG * HD    # 512 feature columns per group
NCORES = 8
NT = T // 128   # 16 t-subtiles
NQ = T // 512   # 4 quarters / q-blocks

f32 = mybir.dt.float32
bf16 = mybir.dt.bfloat16
f8 = mybir.dt.float8e4
Exp = mybir.ActivationFunctionType.Exp
DR = mybir.MatmulPerfMode.DoubleRow
EXP_BIAS = -3.25


def build_nc():
    nc = bacc.Bacc(None, target_bir_lowering=False)
    xt_d = nc.declare_dram_parameter("xt", [DIM, T], bf16, isOutput=False)
    wqk_d = nc.declare_dram_parameter("wqk", [DIM, 2 * CQ], bf16, isOutput=False)
    wv_d = nc.declare_dram_parameter("wv", [DIM, CQ], bf16, isOutput=False)
    wo_d = nc.declare_dram_parameter("wo", [CQ, DIM], bf16, isOutput=False)
    mv_d = nc.declare_dram_parameter("maskv", [128, NT], f32, isOutput=False)
    out_d = nc.declare_dram_parameter("out", [T, DIM], f32, isOutput=True)

    with tile.TileContext(nc) as tc:
        with tc.tile_pool(name="pp", bufs=1) as pp, \
             tc.tile_pool(name="p_p", bufs=3) as p_p, \
             tc.tile_pool(name="p_pb", bufs=3) as p_pb, \
             tc.tile_pool(name="at_p", bufs=2) as at_p, \
             tc.tile_pool(name="dn_p", bufs=2) as dn_p, \
             tc.tile_pool(name="bcs_p", bufs=2) as bcs_p, \
             tc.tile_pool(name="out_p", bufs=2) as out_p, \
             tc.tile_pool(name="dram_p", bufs=2, space="DRAM") as dram_p, \
             tc.tile_pool(name="ps_aux", bufs=2, space="PSUM") as ps_aux, \
             tc.tile_pool(name="ps_s", bufs=2, space="PSUM") as ps_s, \
             tc.tile_pool(name="ps_pv", bufs=1, space="PSUM") as ps_pv:

            # ---- persistent input tiles ----
            xts = [pp.tile([128, T], bf16, name=f"xt{kb}", tag=f"xt{kb}")
                   for kb in range(8)]
            wqk_sb = [pp.tile([128, 2 * CQ], bf16, name=f"wqk{k}", tag=f"wqk{k}")
                      for k in range(8)]
            wv_sb = [pp.tile([128, CQ], bf16, name=f"wv{k}", tag=f"wv{k}")
                     for k in range(8)]
            wo_sb = [pp.tile([128, DIM], bf16, name=f"wo{m}", tag=f"wo{m}")
                     for m in range(4)]
            mv_sb = pp.tile([128, NT], f32, name="maskv_sb", tag="maskv_sb")

            # DMA order: the startup burst (wqk + x quarter 0, needed by the
            # first qk unit) is split across both HWDGE queues; then wv, then
            # the rest on sync.
            for k in range(4):
                nc.sync.dma_start(out=wqk_sb[k],
                                  in_=wqk_d[k * 128:(k + 1) * 128, :])
                nc.scalar.dma_start(out=wqk_sb[4 + k],
                                    in_=wqk_d[(4 + k) * 128:(5 + k) * 128, :])
                nc.sync.dma_start(out=xts[k][:, 0:512],
                                  in_=xt_d[k * 128:(k + 1) * 128, 0:512])
                nc.scalar.dma_start(
                    out=xts[4 + k][:, 0:512],
                    in_=xt_d[(4 + k) * 128:(5 + k) * 128, 0:512])
            for k in range(4):
                nc.sync.dma_start(out=wv_sb[k],
                                  in_=wv_d[k * 128:(k + 1) * 128, :])
                nc.scalar.dma_start(out=wv_sb[4 + k],
                                    in_=wv_d[(4 + k) * 128:(5 + k) * 128, :])
            nc.scalar.dma_start(out=mv_sb, in_=mv_d[:, :])
            for q in range(1, NQ):
                for kb in range(8):
                    nc.sync.dma_start(
                        out=xts[kb][:, q * 512:(q + 1) * 512],
                        in_=xt_d[kb * 128:(kb + 1) * 128, q * 512:(q + 1) * 512])
            for m in range(4):
                nc.sync.dma_start(out=wo_sb[m],
                                  in_=wo_d[m * 128:(m + 1) * 128, :])

            # ---- constants ----
            # one 128x128 causal strip: keep where q_local >= k_local
            dstrip32 = pp.tile([128, 128], f32, name="dstrip32", tag="dstrip32")
            nc.gpsimd.memset(dstrip32, 1.0)
            nc.gpsimd.affine_select(
                out=dstrip32, in_=dstrip32, compare_op=mybir.AluOpType.is_ge,
                fill=0.0, base=0, pattern=[[1, 128]], channel_multiplier=-1)
            dstrip8 = pp.tile([128, 128], f8, name="dstrip8", tag="dstrip8")
            nc.vector.tensor_copy(dstrip8, dstrip32)
            dstripb = pp.tile([128, 128], bf16, name="dstripb", tag="dstripb")
            nc.vector.tensor_copy(dstripb, dstrip32)
            # fp8 V path: values 16x-scaled, ones column 16.0
            ones16 = pp.tile([128, HG], f32, name="ones16", tag="ones16")
            nc.vector.memset(ones16, 16.0)
            ones1 = pp.tile([128, HG], f32, name="ones1", tag="ones1")
            nc.vector.memset(ones1, 1.0)
            mv16_sb = pp.tile([128, NT], f32, name="mv16", tag="mv16")
            nc.vector.tensor_scalar_mul(mv16_sb, mv_sb, 16.0)
            # exp bias (fp8e4m3 range headroom); cancels in the softmax
            nbias = pp.tile([128, 1], f32, name="nbias", tag="nbias")
            nc.vector.memset(nbias, EXP_BIAS)

            # ---- persistent compute tensors ----
            kt = [pp.tile([128, T], bf16, name=f"kt{m}", tag=f"kt{m}")
                  for m in range(4)]
            # fp8 V tiles for DoubleRow: vaug2[kp] holds k-tiles (2kp, 2kp+1)
            # laid out (head, slab, 64 V cols + ones col + 15 pad): the slab
            # stride must be a 16B multiple for dual-fp8 ldweights
            vaug2 = [pp.tile([128, 2 * HG * 80], f8, name=f"va{t}", tag=f"va{t}")
                     for t in range(NT // 2)]
            # bf16 V tiles for q-block 0 (k-tiles 0-3 only), (head, 65) layout
            vaugb = [pp.tile([128, HG * 65], bf16, name=f"vb{t}", tag=f"vb{t}")
                     for t in range(4)]
            qt = [[pp.tile([128, 512], bf16, name=f"qt{q}_{m}", tag=f"qt{q}_{m}")
                   for m in range(4)] for q in range(NQ)]

            ats_cur = {}   # qb -> [4 pair tiles [128, 512] bf16]

            # ---------- qkv quarter units (each ~1.7us of PE) ----------
            def qkv_units(q):
                units = []

                def qk_unit(m):
                    pq = ps_aux.tile([128, 512], f32, name="mm", tag="aux")
                    for kb in range(8):
                        nc.tensor.matmul(
                            pq, wqk_sb[kb][:, m * 128:(m + 1) * 128],
                            xts[kb][:, q * 512:(q + 1) * 512],
                            start=(kb == 0), stop=(kb == 7))
                    if m < 4:
                        nc.vector.tensor_copy(qt[q][m], pq)
                    else:
                        nc.vector.tensor_copy(
                            kt[m - 4][:, q * 512:(q + 1) * 512], pq)
                for m in range(8):
                    units.append(lambda m=m: qk_unit(m))

                def v_unit(ti):
                    pv = ps_aux.tile([128, 512], f32, name="mm", tag="aux")
                    kti = q * 4 + ti
                    for kb in range(8):
                        nc.tensor.matmul(
                            pv, xts[kb][:, kti * 128:(kti + 1) * 128], wv_sb[kb],
                            start=(kb == 0), stop=(kb == 7))
                    pv3 = pv.rearrange("p (h w) -> p h w", w=64)
                    vt4 = vaug2[kti // 2].rearrange(
                        "p (h s u) -> p h s u", s=2, u=80)
                    s = kti % 2
                    nc.vector.tensor_scalar_mul(
                        vt4[:, :, s, 0:64], pv3, mv16_sb[:, kti:kti + 1])
                    nc.vector.tensor_scalar_mul(
                        vt4[:, :, s, 64:65],
                        ones16.rearrange("p (h w) -> p h w", w=1),
                        mv_sb[:, kti:kti + 1])
                    if q == 0:
                        vb3 = vaugb[kti].rearrange("p (h u) -> p h u", u=65)
                        nc.vector.tensor_scalar_mul(
                            vb3[:, :, 0:64], pv3, mv_sb[:, kti:kti + 1])
                        nc.vector.tensor_scalar_mul(
                            vb3[:, :, 64:65],
                            ones1.rearrange("p (h w) -> p h w", w=1),
                            mv_sb[:, kti:kti + 1])
                for ti in range(4):
                    units.append(lambda ti=ti: v_unit(ti))
                return units

            # ---------- out_proj units for one q-block ----------
            def outproj_units(qb):
                units = []

                def op_unit(ti, nb):
                    ats = ats_cur[qb]
                    po = ps_aux.tile([128, 512], f32, name="mm", tag="aux")
                    for m in range(4):
                        nc.tensor.matmul(
                            po, ats[m][:, ti * 128:(ti + 1) * 128],
                            wo_sb[m][:, nb * 512:(nb + 1) * 512],
                            start=(m == 0), stop=(m == 3))
                    ob = out_p.tile([128, 512], f32, name="ob", tag="ob")
                    nc.vector.tensor_copy(ob, po)
                    t0 = (qb * 4 + ti) * 128
                    nc.sync.dma_start(
                        out=out_d[t0:t0 + 128, nb * 512:(nb + 1) * 512], in_=ob)
                for ti in range(4):
                    for nb in range(2):
                        units.append(lambda ti=ti, nb=nb: op_unit(ti, nb))
                return units

            # ---------- attention pair tasks + phase driver ----------
            def att_pair(qb, m, pump):
                nk = 4 * (qb + 1)
                use8 = qb >= 1   # fp8 P@V; q-block 0 stays bf16
                pvp = ps_pv.tile([65, 1024], f32, name="pv", tag="pv")

                def pv_mms(kp, pt2, stop):
                    je = 2 * kp - 4 * qb
                    pt4 = pt2.rearrange("p (h s w) -> p h s w", s=2, w=512)
                    va4 = vaug2[kp].rearrange("p (h s u) -> p h s u", s=2, u=80)
                    for h in range(2):
                        if use8 and je < 0:
                            nc.tensor.matmul(
                                pvp[:, h * 512:(h + 1) * 512],
                                va4[:, 2 * m + h, :, 0:65], pt4[:, h, :, :],
                                start=(kp == 0), stop=stop, perf_mode=DR)
                        else:
                            for s in range(2):
                                w0 = 128 * (je + s) if je + s > 0 else 0
                                lhs = (va4[:, 2 * m + h, s, 0:65] if use8 else
                                       vaugb[2 * kp + s]
                                       [:, (2 * m + h) * 65:(2 * m + h + 1) * 65])
                                nc.tensor.matmul(
                                    pvp[:, h * 512 + w0:(h + 1) * 512],
                                    lhs, pt4[:, h, s, w0:512],
                                    start=(kp == 0 and s == 0),
                                    stop=(stop and s == 1))

                prev = None
                for kp in range(nk // 2):
                    pt2 = ((p_p if use8 else p_pb)
                           .tile([128, 2048], f8 if use8 else bf16,
                                 name="p", tag="p"))
                    pt4 = pt2.rearrange("p (h s w) -> p h s w", s=2, w=512)
                    dstrip = dstrip8 if use8 else dstripb
                    for s in range(2):
                        kti = 2 * kp + s
                        j = kti - 4 * qb
                        w0 = 128 * j if j > 0 else 0
                        sp = ps_s.tile([128, 1024], f32, name="s", tag="s")
                        nc.tensor.matmul(
                            sp[:, w0:512],
                            kt[m][0:64, kti * 128:(kti + 1) * 128],
                            qt[qb][m][0:64, w0:512], start=True, stop=True)
                        nc.tensor.matmul(
                            sp[:, 512 + w0:1024],
                            kt[m][64:128, kti * 128:(kti + 1) * 128],
                            qt[qb][m][64:128, w0:512], start=True, stop=True)
                        pump()
                        s3 = sp.rearrange("p (h w) -> p h w", w=512)
                        nc.scalar.activation(
                            pt4[:, :, s, w0:512], s3[:, :, w0:512], Exp,
                            scale=0.125, bias=nbias[:, 0:1])
                        if j >= 0:
                            for h in range(2):
                                nc.vector.tensor_mul(
                                    pt4[:, h, s, w0:w0 + 128],
                                    pt4[:, h, s, w0:w0 + 128], dstrip)
                        if s == 0 and prev is not None:
                            pv_mms(*prev, stop=False)
                        pump()
                    prev = (kp, pt2)
                pv_mms(*prev, stop=True)

                # evacuate pvp fast (~2us) so ps_pv (bufs=1) recycles
                dn = dn_p.tile([1, 1024], f32, name="dn", tag="dn")
                nc.vector.tensor_copy(dn, pvp[64:65, 0:1024])
                araw = at_p.tile([128, 512], f32, name=f"ar{m}", tag=f"ar{m}")
                nc.vector.tensor_copy(araw[0:64, :], pvp[0:64, 0:512])
                nc.vector.tensor_copy(araw[64:128, :], pvp[0:64, 512:1024])
                # reciprocal at 8 elems/lane via DRAM reshape (a [1,1024]
                # reciprocal costs ~7.9us on DVE: time scales with free size);
                # round-trip DMAs ride the idle gpsimd SWDGE queue.
                dd = dram_p.tile([1, 1024], f32, name="dd", tag="dd")
                nc.gpsimd.dma_start(out=dd, in_=dn)
                den128 = dn_p.tile([128, 8], f32, name="den128", tag="den128")
                nc.gpsimd.dma_start(
                    out=den128,
                    in_=dd.rearrange("i w -> (i w)").rearrange(
                        "(p c) -> p c", c=8))
                rec128 = dn_p.tile([128, 8], f32, name="rec128", tag="rec128")
                nc.vector.reciprocal(rec128, den128)
                # cast to bf16 on the casting SWDGE store (half the broadcast
                # bytes); broadcasts ride the scalar HWDGE queue (low latency)
                dd2 = dram_p.tile([1, 1024], bf16, name="dd2", tag="dd2")
                nc.gpsimd.dma_start(
                    out=dd2.rearrange("i w -> (i w)").rearrange(
                        "(p c) -> p c", c=8),
                    in_=rec128)
                bcs = bcs_p.tile([128, 512], bf16, name="bcs", tag="bcs")
                for h in range(2):
                    nc.scalar.dma_start(
                        out=bcs[h * 64:(h + 1) * 64, :],
                        in_=dd2[0:1, h * 512:(h + 1) * 512]
                        .partition_broadcast(64))
                atm = at_p.tile([128, 512], bf16, name=f"at{m}", tag=f"at{m}")
                ats_cur[qb][m] = atm
                nc.vector.tensor_mul(
                    atm[0:64, :], araw[0:64, :], bcs[0:64, :])
                nc.vector.tensor_mul(
                    atm[64:128, :], araw[64:128, :], bcs[64:128, :])

            def run_phase(tasks, fillers, n_units):
                """tasks: closures taking pump(); fillers pumped proportionally."""
                nf = len(fillers)
                state = {"fi": 0, "ai": 0}

                def pump():
                    state["ai"] += 1
                    while state["fi"] * n_units < state["ai"] * nf \
                            and state["fi"] < nf:
                        fillers[state["fi"]]()
                        state["fi"] += 1
                for t in tasks:
                    t(pump)
                while state["fi"] < nf:
                    fillers[state["fi"]]()
                    state["fi"] += 1

            # ---------------- emission schedule ----------------
            for u in qkv_units(0):
                u()
            for qb in range(NQ):
                ats_cur[qb] = [None] * 4

            def phase_tasks(qb):
                def mk(m):
                    def t(pump):
                        att_pair(qb, m, pump)
                    return t
                return [mk(m) for m in range(4)]

            # phase 1: att(0) + qkv(1)
            run_phase(phase_tasks(0), qkv_units(1), 32)
            # phase 2: att(1) + qkv(2) + op(0)
            run_phase(phase_tasks(1), qkv_units(2) + outproj_units(0), 64)
            # phase 3: att(2) + att(3) pairs 0-1, fillers qkv(3) + op(1);
            # fillers exhaust during att(2) so att(3) has its inputs emitted
            run_phase(phase_tasks(2) + phase_tasks(3)[:2],
                      qkv_units(3) + outproj_units(1), 96)
            # phase 4: att(3) pairs 2-3 + op(2)
            run_phase(phase_tasks(3)[2:], outproj_units(2), 64)
            for u in outproj_units(NQ - 1):
                u()
    nc.finalize()
    return nc


_NC_CACHE = {}


def _get_nc():
    if "nc" not in _NC_CACHE:
        _NC_CACHE["nc"] = build_nc()
    return _NC_CACHE["nc"]


def _make_in_maps(x, w_qkv, w_out, attn_mask):
    x = np.asarray(x, dtype=np.float32)
    w_qkv = np.asarray(w_qkv, dtype=np.float32)
    w_out = np.asarray(w_out, dtype=np.float32)
    am = np.asarray(attn_mask)
    bf = ml_dtypes.bfloat16
    in_maps = []
    for c in range(NCORES):
        b, hg = c // 2, c % 2
        wqk_c = np.ascontiguousarray(np.concatenate(
            [w_qkv[:, hg * CQ:(hg + 1) * CQ],
             w_qkv[:, DIM + hg * CQ:DIM + (hg + 1) * CQ]], axis=1)).astype(bf)
        wv_c = np.ascontiguousarray(
            w_qkv[:, 2 * DIM + hg * CQ:2 * DIM + (hg + 1) * CQ]).astype(bf)
        wo_c = np.ascontiguousarray(w_out[hg * CQ:(hg + 1) * CQ, :]).astype(bf)
        mv_c = np.ascontiguousarray(
            am[b].astype(np.float32).reshape(NT, 128).T)
        xt_c = np.ascontiguousarray(x[b].T).astype(bf)
        in_maps.append({
            "xt": xt_c,
            "wqk": wqk_c,
            "wv": wv_c,
            "wo": wo_c,
            "maskv": mv_c,
        })
    return in_maps


def run(x, w_qkv, w_out, attn_mask, trace=False):
    nc = _get_nc()
    in_maps = _make_in_maps(x, w_qkv, w_out, attn_mask)
    res = run_bass_kernel_spmd(nc, in_maps, list(range(NCORES)), trace=trace)
    outs = [res.results[c]["out"] for c in range(NCORES)]
    full = np.stack([outs[2 * b] + outs[2 * b + 1] for b in range(B)], axis=0)
    return full.astype(np.float32), res


def kernel(x, w_qkv, w_out, attn_mask):
    full, _ = run(x, w_qkv, w_out, attn_mask, trace=False)
    return full
